# revision 12
# baseline (speedup 1.0000x reference)
"""Adversarial-MMD loss (nn_Advmmd) on 8 Trainium2 NeuronCores via Bass/Tile.

Math (eval mode, lamb=0):
  adv:  the discriminator is Linear(2048,128) -> Dropout(eval) -> Linear(128,1)
        with NO nonlinearity, so it collapses to a single linear functional
        z = x.w + beta with w = W2@W1 [2048], beta = W2@b1 + b2.
        adv_loss = 0.5*(mean log(1+exp(-z_src)) + mean log(1+exp(+z_tgt)))
  mmd:  total = [source;target] [8192,2048]; L2_ij = sq_i + sq_j - 2 G_ij with
        G = total@total.T;  bandwidth bw = sum(L2)/(n^2-n)/4 where
        sum(L2) = 2n*sum(sq) - 2*||sum_j total_j||^2 (exact identity);
        K = sum_{p=0..4} exp(-L2/(bw*2^p));
        loss = mean K[XX] + mean K[YY] - mean K[XY] - mean K[YX].

Distribution: data-parallel over Gram rows. Core c owns 1024 rows; computes
its [1024, 8192] Gram block in bf16 on the PE (fp32 accumulate), applies the
five Gaussian kernels on the Scalar engine (exp with per-partition
scale/bias; row-sums come for free via accum_out), and reduces to two
scalars (left-half / right-half block sums). Bandwidth statistics are
computed on-device (per-core row norms + column sums, combined with one
AllGather + one AllReduce). Host only shards inputs and sums 8x3 scalars.
"""

import numpy as np
import ml_dtypes

N_CORES = 8
B = 4096
D = 2048
NT = 2 * B            # 8192 total rows
RPC = NT // N_CORES   # 1024 rows per core
MT = RPC // 128       # 8 m-tiles per core
KT = D // 128         # 16 k-tiles
NCHUNK = 512          # columns per n-chunk (one psum bank)
NN = NT // NCHUNK     # 16 n-chunks
KERNEL_NUM = 5

_cached = {}


def _build_module(blocks=None):
    blocks = blocks or {'const','disc','rows','cc','bw','adv','main','epi'}
    import concourse.bass as bass
    import concourse.tile as tile
    import concourse.mybir as mybir
    from concourse import bacc

    f32 = mybir.dt.float32
    bf16 = mybir.dt.bfloat16
    AF = mybir.ActivationFunctionType
    ALU = mybir.AluOpType

    nc = bacc.Bacc(
        "TRN2",
        target_bir_lowering=False,
        debug=False,
        enable_asserts=False,
        num_devices=N_CORES,
    )

    # ---- kernel I/O (per-core shapes; data differs per core) ----
    totalT_d = nc.dram_tensor("totalT", [D, NT], bf16, kind="ExternalInput")
    blockT_d = nc.dram_tensor("blockT", [D, RPC], bf16, kind="ExternalInput")
    rows_d = nc.dram_tensor("rows", [RPC, D], bf16, kind="ExternalInput")
    W1_d = nc.dram_tensor("w1", [128, D], f32, kind="ExternalInput")
    W2T_d = nc.dram_tensor("w2t", [128, 1], f32, kind="ExternalInput")
    b1_d = nc.dram_tensor("b1c", [128, 1], f32, kind="ExternalInput")
    b2_d = nc.dram_tensor("b2c", [1, 1], f32, kind="ExternalInput")
    sgn_d = nc.dram_tensor("sgn", [128, 1], f32, kind="ExternalInput")
    hot_d = nc.dram_tensor("hot", [128, N_CORES], f32, kind="ExternalInput")
    lad_d = nc.dram_tensor("ladder", [1, KERNEL_NUM], f32, kind="ExternalInput")
    out_d = nc.dram_tensor("out", [1, 8], f32, kind="ExternalOutput")

    rg = [list(range(N_CORES))]

    with tile.TileContext(nc) as tc:
        with (
            tc.tile_pool(name="big", bufs=1) as big,
            tc.tile_pool(name="rhsp", bufs=4) as rhsp,
            tc.tile_pool(name="x2p", bufs=4) as x2p,
            tc.tile_pool(name="ejp", bufs=2) as ejp,
            tc.tile_pool(name="smalls", bufs=1) as smalls,
            tc.tile_pool(name="gpsum", bufs=3, space="PSUM") as gpsum,
            tc.tile_pool(name="spsum", bufs=1, space="PSUM") as spsum,
            tc.tile_pool(name="dram", bufs=1, space="DRAM") as dram,
            tc.tile_pool(name="prol", bufs=1) as prol,
            tc.tile_pool(name="rowp", bufs=3) as rowp,
        ):
            # ---------- persistent tiles ----------
            lhsT_all = big.tile([128, KT, RPC], bf16)       # 32KB/p
            colsq = big.tile([128, NT], f32)                # 32KB/p
            accs = big.tile([128, (NN // 2) * MT * KERNEL_NUM], f32)
            rowsq = smalls.tile([128, MT], f32)
            bias_all = smalls.tile([128, KERNEL_NUM * MT], f32)
            scales5 = smalls.tile([128, KERNEL_NUM], f32)
            vec5b = smalls.tile([128, KERNEL_NUM], f32)
            zcols = smalls.tile([128, MT], f32)
            ecols = smalls.tile([128, MT], f32)
            lncols = smalls.tile([128, MT], f32)
            fin = smalls.tile([128, 4], f32)
            ones_bf = smalls.tile([128, 1], bf16)
            ones_f = smalls.tile([128, 1], f32)
            sgn_sb = smalls.tile([128, 1], f32)
            hot_sb = smalls.tile([128, N_CORES], f32)
            lad_sb = smalls.tile([1, KERNEL_NUM], f32)
            vec5 = smalls.tile([1, KERNEL_NUM], f32)
            sqsum = smalls.tile([1, 1], f32)
            s2s = smalls.tile([1, 1], f32)
            t1s = smalls.tile([1, 1], f32)
            bws = smalls.tile([1, 1], f32)
            bwinv = smalls.tile([1, 1], f32)
            beta_sb = smalls.tile([1, 1], f32)
            betab = smalls.tile([128, 1], f32)
            outsb = smalls.tile([4, 1], f32)

            # ---------- prologue tiles ----------
            W1_sb = prol.tile([128, D], f32)                # 8KB/p
            W2T_sb = prol.tile([128, 1], f32)
            b1_sb = prol.tile([128, 1], f32)
            b2_sb = prol.tile([1, 1], f32)
            wb = prol.tile([128, D], f32)                   # 8KB/p
            w_sb = prol.tile([1, D], f32)                   # 8KB/p
            s_sb = prol.tile([128, KT], f32)
            s_sb2 = prol.tile([128, KT], f32)
            sprod = prol.tile([128, KT], f32)
            sprod_r = prol.tile([128, 1], f32)
            junk2 = prol.tile([128, D], bf16)               # 4KB/p
            junk3 = prol.tile([128, D], f32)                # 8KB/p

            # ---------- DRAM collective buffers ----------
            # one packed AllReduce buffer: [NT] sq slots (disjoint per core,
            # zeros elsewhere -> concat under add) then [D] s (true sum)
            cc_in = dram.tile([NT + D], f32)
            cc_out = dram.tile([NT + D], f32, addr_space="Shared")

            # ---------- constant loads ----------
            nc.sync.dma_start(
                lhsT_all[:],
                blockT_d.ap().rearrange("(kt p) m -> p kt m", p=128),
            )
            nc.gpsimd.dma_start(W1_sb[:], W1_d[:, :])
            nc.gpsimd.dma_start(W2T_sb[:], W2T_d[:, :])
            nc.gpsimd.dma_start(b1_sb[:], b1_d[:, :])
            nc.gpsimd.dma_start(b2_sb[:], b2_d[:, :])
            nc.gpsimd.dma_start(sgn_sb[:], sgn_d[:, :])
            nc.gpsimd.dma_start(hot_sb[:], hot_d[:, :])
            nc.gpsimd.dma_start(lad_sb[:], lad_d[:, :])
            nc.vector.memset(ones_bf[:], 1.0)
            nc.vector.memset(ones_f[:], 1.0)
            nc.vector.memset(fin[:, 3:4], 0.0)

            # ---------- discriminator collapse: w = W2@W1, beta = W2@b1+b2 ----
            if 'disc' not in blocks:
                nc.vector.memset(wb[:], 0.0)
                nc.vector.memset(betab[:], 0.0)
                nc.vector.memset(beta_sb[:], 0.0)
            if 'disc' in blocks:
              for ch in range(4):
                w_ps = spsum.tile([1, 512], f32, tag="sps")
                nc.tensor.matmul(
                    w_ps[:],
                    W2T_sb[:],
                    W1_sb[:, bass.ts(ch, 512)],
                    start=True,
                    stop=True,
                )
                nc.scalar.copy(w_sb[:, bass.ts(ch, 512)], w_ps[:])

              beta_ps = spsum.tile([1, 1], f32, tag="sps")
              nc.tensor.matmul(beta_ps[:], W2T_sb[:], b1_sb[:], start=True, stop=True)
              nc.vector.tensor_scalar_add(beta_sb[:], beta_ps[:], b2_sb[0:1, 0:1])
              nc.gpsimd.partition_broadcast(betab[:], beta_sb[:])
              nc.gpsimd.partition_broadcast(wb[:], w_sb[:])

            # ---------- per-row-tile stats: sq, z, column-sum ----------
            if 'rows' not in blocks:
                nc.vector.memset(rowsq[:], 1000.0)
                nc.vector.memset(zcols[:], 0.0)
                nc.vector.memset(s_sb[:], 0.0)
            s_ps = spsum.tile([128, KT], f32, tag="sps")
            zraw = smalls.tile([128, MT], f32)
            for mt in range(MT if 'rows' in blocks else 0):
                rt = rowp.tile([128, D], bf16)
                nc.gpsimd.dma_start(rt[:], rows_d[bass.ts(mt, 128), :])
                # sq_i = sum_k x~^2 via ACT Square + free-axis accumulate
                nc.scalar.activation(
                    junk2[:], rt[:], AF.Square, bias=0.0, scale=1.0,
                    accum_out=rowsq[:, mt : mt + 1],
                )
                # z_i = sum_k x~ * w (beta added below)
                nc.vector.tensor_tensor(
                    out=junk3[:], in0=rt[:], in1=wb[:], op=ALU.mult
                )
                nc.vector.tensor_reduce(
                    zraw[:, mt : mt + 1], junk3[:],
                    axis=mybir.AxisListType.X, op=ALU.add,
                )
                for kt in range(KT):
                    nc.tensor.matmul(
                        s_ps[:, kt : kt + 1],
                        rt[:, bass.ts(kt, 128)],
                        ones_bf[:],
                        start=(mt == 0),
                        stop=(mt == MT - 1),
                    )
            if 'rows' in blocks:
                nc.scalar.copy(s_sb[:], s_ps[:])
                nc.vector.tensor_scalar_add(zcols[:], zraw[:], betab[:])

            # ---------- adv partial: sum log(1+exp(sgn*z)) ----------
            if 'adv' in blocks:
                nc.scalar.activation(
                    ecols[:], zcols[:], AF.Exp, bias=0.0, scale=sgn_sb[:]
                )
                nc.scalar.activation(
                    lncols[:], ecols[:], AF.Ln, bias=1.0, scale=1.0,
                    accum_out=fin[:, 2:3],
                )
            else:
                nc.vector.memset(fin[:, 2:3], 0.0)

            # ---------- collectives: sq AllGather + s AllReduce ----------
            if 'cc' in blocks:
                # zero the sq region, then place our slots via the iota-mask
                # trick: per-core row offset comes from per-core input `cofs`
                # (a [1,1] int offset is not expressible; instead each core
                # scatters to its own slot range using the cofs-th offset via
                # a dynamic DMA).  Simpler: zero everything, write own slots
                # at a per-core DRAM offset carried by an input-driven DMA is
                # not available -> use per-core input tensor `slotmask`?  We
                # avoid all of that: each core writes its rowsq into the slot
                # range [cofs, cofs+RPC) where cofs is baked per-core into
                # the `sqslot` input AP offset... SPMD forbids per-core code,
                # so instead write rowsq to ALL slots scaled by a per-core
                # one-hot [8] mask via 8 small DMAs?  Cheapest correct SPMD
                # scheme: scatter rowsq*(mask_c) for each of the 8 slot
                # ranges, where mask_c is a per-core {0,1} input scalar
                # multiplied on DVE first.
                zer = prol.tile([128, (NT + D) // 128], f32)
                nc.vector.memset(zer[:], 0.0)
                nc.gpsimd.dma_start(
                    cc_in.rearrange("(p m) -> p m", p=128), zer[:]
                )
                for c8 in range(N_CORES):
                    mrq = prol.tile([128, MT], f32, tag="mrq", bufs=2)
                    nc.vector.tensor_scalar_mul(
                        mrq[:], rowsq[:], hot_sb[:, c8 : c8 + 1]
                    )
                    nc.gpsimd.dma_start(
                        cc_in[c8 * RPC : (c8 + 1) * RPC].rearrange(
                            "(m p) -> p m", p=128
                        ),
                        mrq[:],
                    )
                nc.gpsimd.dma_start(
                    cc_in[NT : NT + D].rearrange("(kt p) -> p kt", p=128),
                    s_sb[:],
                )
                nc.gpsimd.collective_compute(
                    "AllReduce",
                    ALU.add,
                    replica_groups=rg,
                    ins=[cc_in.opt()],
                    outs=[cc_out.opt()],
                )
                # broadcast sq over partitions; raw first, then scale by -1/2
                nc.gpsimd.dma_start(
                    colsq[:], cc_out[None, 0:NT].broadcast_to((128, NT))
                )
                nc.gpsimd.dma_start(
                    s_sb2[:], cc_out[NT : NT + D].rearrange("(kt p) -> p kt", p=128)
                )
            else:
                nc.vector.memset(colsq[:], 1000.0)
                nc.vector.memset(s_sb2[:], 0.0)

            # ---------- bandwidth ----------
            if 'bw' not in blocks:
                nc.vector.memset(sqsum[:], 0.0)
                nc.vector.memset(s2s[:], 0.0)
                nc.vector.memset(bws[:], 1000.0)
                nc.vector.memset(scales5[:], 0.001)
                nc.vector.memset(bias_all[:], -1.0)
            if 'bw' in blocks:
              nc.vector.tensor_reduce(
                sqsum[:], colsq[0:1, :], axis=mybir.AxisListType.X, op=ALU.add
              )
              nc.vector.tensor_scalar_mul(colsq[:], colsq[:], -0.5)
              nc.vector.tensor_tensor(
                out=sprod[:], in0=s_sb2[:], in1=s_sb2[:], op=ALU.mult
              )
              nc.vector.tensor_reduce(
                sprod_r[:], sprod[:], axis=mybir.AxisListType.X, op=ALU.add
              )
              s2_ps = spsum.tile([1, 1], f32, tag="sps")
              nc.tensor.matmul(s2_ps[:], sprod_r[:], ones_f[:], start=True, stop=True)
              nc.scalar.copy(s2s[:], s2_ps[:])
              denom = float(NT) * float(NT) - float(NT)
              a_const = float(2.0 * NT / (4.0 * denom))
              b_const = float(-2.0 / (4.0 * denom))
              nc.vector.tensor_scalar_mul(t1s[:], sqsum[:], a_const)
              nc.vector.tensor_scalar(
                out=bws[:], in0=s2s[:], scalar1=b_const, scalar2=t1s[0:1, 0:1],
                op0=ALU.mult, op1=ALU.add,
              )
              nc.vector.reciprocal(bwinv[:], bws[:])
              nc.vector.tensor_scalar_mul(vec5[:], lad_sb[:], bwinv[0:1, 0:1])
              nc.gpsimd.partition_broadcast(vec5b[:], vec5[:])
              nc.vector.tensor_scalar_mul(scales5[:], vec5b[:], 2.0)
              for p in range(KERNEL_NUM):
                nc.vector.tensor_scalar(
                    out=bias_all[:, bass.ts(p, MT)],
                    in0=rowsq[:],
                    scalar1=vec5b[:, p : p + 1],
                    scalar2=-1.0,
                    op0=ALU.mult,
                    op1=ALU.mult,
                )

            # ---------- main loop: Gram block + 5 kernels + row sums ----------
            if 'main' not in blocks or 'main1' in blocks:
                nc.vector.memset(accs[:], 0.0)
            totalT_r = totalT_d.ap().rearrange("(kt p) n -> p kt n", p=128)
            NPAIR = NN // 2
            n_pairs = NPAIR if 'main' in blocks else (1 if 'main1' in blocks else 0)
            for n2 in range(n_pairs):
                rhs_a = rhsp.tile([128, KT, NCHUNK], bf16, tag="rhs")
                nc.sync.dma_start(rhs_a[:], totalT_r[:, :, bass.ts(2 * n2, NCHUNK)])
                rhs_b = rhsp.tile([128, KT, NCHUNK], bf16, tag="rhs")
                nc.sync.dma_start(
                    rhs_b[:], totalT_r[:, :, bass.ts(2 * n2 + 1, NCHUNK)]
                )
                for m in range(MT):
                    gt = gpsum.tile([128, 2 * NCHUNK], f32)
                    for k in range(KT):
                        lw = lhsT_all[:, k, bass.ts(m, 128)]
                        nc.tensor.matmul(
                            gt[:, 0:NCHUNK], lw, rhs_a[:, k, :],
                            start=(k == 0), stop=(k == KT - 1),
                        )
                        nc.tensor.matmul(
                            gt[:, NCHUNK : 2 * NCHUNK], lw, rhs_b[:, k, :],
                            start=(k == 0), stop=(k == KT - 1),
                        )
                    x2 = x2p.tile([128, 2 * NCHUNK], f32)
                    nc.vector.tensor_add(
                        x2[:], gt[:], colsq[:, bass.ts(n2, 2 * NCHUNK)]
                    )
                    for p in range(KERNEL_NUM):
                        ej = ejp.tile([128, 2 * NCHUNK], f32)
                        slot = (n2 * MT + m) * KERNEL_NUM + p
                        nc.scalar.activation(
                            ej[:],
                            x2[:],
                            AF.Exp,
                            bias=bias_all[:, p * MT + m : p * MT + m + 1],
                            scale=scales5[:, p : p + 1],
                            accum_out=accs[:, slot : slot + 1],
                        )

            # ---------- epilogue: block sums -> scalars ----------
            if 'epi' not in blocks:
                nc.vector.memset(fin[:, 0:2], 0.0)
            half = (NN // 4) * MT * KERNEL_NUM
            if 'epi' in blocks:
              nc.vector.tensor_reduce(
                fin[:, 0:1], accs[:, :half], axis=mybir.AxisListType.X, op=ALU.add
              )
              nc.vector.tensor_reduce(
                fin[:, 1:2], accs[:, half:], axis=mybir.AxisListType.X, op=ALU.add
              )
            fin_ps = spsum.tile([4, 1], f32, tag="sps")
            nc.tensor.matmul(fin_ps[:], fin[:], ones_f[:], start=True, stop=True)
            nc.scalar.copy(outsb[:], fin_ps[:])
            nc.sync.dma_start(out_d[0, 0:4], outsb[:])
            nc.sync.dma_start(out_d[0, 4:5], bws[:])
            nc.sync.dma_start(out_d[0, 5:6], sqsum[:])
            nc.sync.dma_start(out_d[0, 6:7], s2s[:])
            nc.sync.dma_start(out_d[0, 7:8], beta_sb[:])

    nc.compile()
    return nc


def _get_module():
    import os

    bl = os.environ.get("ADVMMD_BLOCKS", "")
    blocks = set(b for b in bl.split(",") if b) or None
    key = ("nc", bl)
    if key not in _cached:
        _cached[key] = _build_module(blocks)
    return _cached[key]


def kernel(source, target, W1, b1, W2, b2, _trace=False, _trace_kwargs=None):
    import concourse.bass_utils as bass_utils

    nc = _get_module()

    total = np.concatenate(
        [np.asarray(source, np.float32), np.asarray(target, np.float32)], axis=0
    )
    tot_bf = total.astype(ml_dtypes.bfloat16)
    totalT = np.ascontiguousarray(tot_bf.T)  # [D, NT] bf16, shared

    W1f = np.asarray(W1, np.float32)
    W2f = np.asarray(W2, np.float32)
    b1f = np.asarray(b1, np.float32).reshape(128, 1)
    b2f = np.asarray(b2, np.float32).reshape(1, 1)
    W2T = np.ascontiguousarray(W2f.reshape(1, 128).T)  # [128,1]
    ladder = (0.5 ** np.arange(KERNEL_NUM, dtype=np.float32)).reshape(1, -1)

    in_maps = []
    for c in range(N_CORES):
        rows = np.ascontiguousarray(tot_bf[c * RPC : (c + 1) * RPC])
        blockT = np.ascontiguousarray(totalT[:, c * RPC : (c + 1) * RPC])
        sgn = np.full((128, 1), -1.0 if c < N_CORES // 2 else 1.0, np.float32)
        hot = np.zeros((128, N_CORES), np.float32)
        hot[:, c] = 1.0
        in_maps.append(
            {
                "totalT": totalT,
                "blockT": blockT,
                "rows": rows,
                "w1": W1f,
                "w2t": W2T,
                "b1c": b1f,
                "b2c": b2f,
                "sgn": sgn,
                "hot": hot,
                "ladder": ladder,
            }
        )

    kwargs = dict(_trace_kwargs or {})
    res = bass_utils.run_bass_kernel_spmd(
        nc, in_maps, core_ids=list(range(N_CORES)), trace=_trace, **kwargs
    )
    outs = [r["out"][0] for r in res.results]

    SL = [float(o[0]) for o in outs]  # per-core sum over left half (j < B)
    SR = [float(o[1]) for o in outs]  # per-core sum over right half (j >= B)
    SA = [float(o[2]) for o in outs]  # per-core adv partial

    h = N_CORES // 2
    sxx = sum(SL[:h])
    syx = sum(SL[h:])
    sxy = sum(SR[:h])
    syy = sum(SR[h:])
    loss = np.float32((sxx + syy - sxy - syx) / (float(B) * float(B)))
    adv = np.float32(sum(SA) / float(NT))

    if _trace:
        kernel._last_results = res
    return (np.asarray(loss, np.float32), np.asarray(adv, np.float32))


# revision 16
# speedup vs baseline: 1.1259x; 1.1259x over previous
"""Adversarial-MMD loss (nn_Advmmd) on 8 Trainium2 NeuronCores via Bass/Tile.

Math (eval mode, lamb=0):
  adv:  the discriminator is Linear(2048,128) -> Dropout(eval) -> Linear(128,1)
        with NO nonlinearity, so it collapses to a single linear functional
        z = x.w + beta with w = W2@W1 [2048], beta = W2@b1 + b2.
        adv_loss = 0.5*(mean log(1+exp(-z_src)) + mean log(1+exp(+z_tgt)))
  mmd:  total = [source;target] [8192,2048]; L2_ij = sq_i + sq_j - 2 G_ij with
        G = total@total.T;  bandwidth bw = sum(L2)/(n^2-n)/4 where
        sum(L2) = 2n*sum(sq) - 2*||sum_j total_j||^2 (exact identity);
        K = sum_{p=0..4} exp(-L2/(bw*2^p));
        loss = mean K[XX] + mean K[YY] - mean K[XY] - mean K[YX].

Distribution: data-parallel over Gram rows.  Core c owns 1024 rows; it
computes its [1024, 8192] Gram block in bf16 on the PE (fp32 accumulate),
applies the five Gaussian kernels on the Scalar engine (exp with
per-partition scale/bias; row sums come free via accum_out) and reduces to
two scalars (left/right half block sums).  Row norms and column sums are
exchanged with a single small AllGather; everything else is local.

The host pre-permutes rows inside each core block ((m,p) -> (p,m) with
p=partition) so the on-device stats land contiguously in DRAM for the
collective; all Gram/block sums are permutation invariant.

The pipeline drains PE's PSUM through a plain copy into SBUF tiles, so the
matmul stream never waits for the collective; the -sq_j/2 column correction
is added in-place afterwards, once the AllGather lands.
"""

import numpy as np
import ml_dtypes

N_CORES = 8
B = 4096
D = 2048
NT = 2 * B            # 8192 total rows
RPC = NT // N_CORES   # 1024 rows per core
MT = RPC // 128       # 8 m-tiles per core
KT = D // 128         # 16 k-tiles
NCHUNK = 512
NPAIR = NT // (2 * NCHUNK)   # 8 pairs of 512-column chunks
KERNEL_NUM = 5
CCW = RPC + D         # per-core AllGather payload: [sq (1024) | s (2048)]

_cached = {}


def _build_module():
    import concourse.bass as bass
    import concourse.tile as tile
    import concourse.mybir as mybir
    from concourse import bacc

    f32 = mybir.dt.float32
    bf16 = mybir.dt.bfloat16
    AF = mybir.ActivationFunctionType
    ALU = mybir.AluOpType

    nc = bacc.Bacc(
        "TRN2",
        target_bir_lowering=False,
        debug=False,
        enable_asserts=False,
        num_devices=N_CORES,
    )

    totalT_d = nc.dram_tensor("totalT", [D, NT], bf16, kind="ExternalInput")
    blockT_d = nc.dram_tensor("blockT", [D, RPC], bf16, kind="ExternalInput")
    rows_d = nc.dram_tensor("rows", [RPC, D], bf16, kind="ExternalInput")
    W1_d = nc.dram_tensor("w1", [128, D], f32, kind="ExternalInput")
    W2T_d = nc.dram_tensor("w2t", [128, 1], f32, kind="ExternalInput")
    b1_d = nc.dram_tensor("b1c", [128, 1], f32, kind="ExternalInput")
    b2_d = nc.dram_tensor("b2c", [1, 1], f32, kind="ExternalInput")
    sgn_d = nc.dram_tensor("sgn", [128, 1], f32, kind="ExternalInput")
    lad_d = nc.dram_tensor("ladder", [1, KERNEL_NUM], f32, kind="ExternalInput")
    out_d = nc.dram_tensor("out", [1, 8], f32, kind="ExternalOutput")

    rg = [list(range(N_CORES))]

    with tile.TileContext(nc) as tc:
        with (
            tc.tile_pool(name="big", bufs=1) as big,
            tc.tile_pool(name="rhsp", bufs=4) as rhsp,
            tc.tile_pool(name="x2p", bufs=6) as x2p,
            tc.tile_pool(name="ejp", bufs=2) as ejp,
            tc.tile_pool(name="smalls", bufs=1) as smalls,
            tc.tile_pool(name="gpsum", bufs=3, space="PSUM") as gpsum,
            tc.tile_pool(name="spsum", bufs=1, space="PSUM") as spsum,
            tc.tile_pool(name="dram", bufs=1, space="DRAM") as dram,
            tc.tile_pool(name="prol", bufs=1) as prol,
            tc.tile_pool(name="rowp", bufs=2) as rowp,
        ):
            # ---------- persistent tiles ----------
            lhsT_all = big.tile([128, KT, RPC], bf16)       # 32KB/p
            colsq = big.tile([128, NT], f32)                # 32KB/p
            accs = big.tile([128, NPAIR * MT * KERNEL_NUM], f32)
            rowsq = smalls.tile([128, MT], f32)
            bias_all = smalls.tile([128, KERNEL_NUM * MT], f32)
            scales5 = smalls.tile([128, KERNEL_NUM], f32)
            vec5b = smalls.tile([128, KERNEL_NUM], f32)
            zcols = smalls.tile([128, MT], f32)
            zraw = smalls.tile([128, MT], f32)
            ecols = smalls.tile([128, MT], f32)
            lncols = smalls.tile([128, MT], f32)
            fin = smalls.tile([128, 4], f32)
            ones_bf = smalls.tile([128, 1], bf16)
            ones_f = smalls.tile([128, 1], f32)
            sgn_sb = smalls.tile([128, 1], f32)
            lad_sb = smalls.tile([1, KERNEL_NUM], f32)
            vec5 = smalls.tile([1, KERNEL_NUM], f32)
            sqsum = smalls.tile([1, 1], f32)
            s2s = smalls.tile([1, 1], f32)
            t1s = smalls.tile([1, 1], f32)
            bws = smalls.tile([1, 1], f32)
            bwinv = smalls.tile([1, 1], f32)
            beta_sb = smalls.tile([1, 1], f32)
            betab = smalls.tile([128, 1], f32)
            outsb = smalls.tile([4, 1], f32)

            # ---------- prologue tiles ----------
            W1_sb = prol.tile([128, D], f32)                # 8KB/p
            W2T_sb = prol.tile([128, 1], f32)
            b1_sb = prol.tile([128, 1], f32)
            b2_sb = prol.tile([1, 1], f32)
            wb = prol.tile([128, D], bf16)                  # 4KB/p
            w_sb = prol.tile([1, D], bf16)
            s2d_sb = prol.tile([128, KT], f32)
            s3v = prol.tile([128, N_CORES, KT], f32)
            s_glob = prol.tile([128, KT], f32)
            sprod = prol.tile([128, KT], f32)
            sprod_r = prol.tile([128, 1], f32)
            junk2 = prol.tile([128, D], bf16)               # 4KB/p
            junk3 = prol.tile([128, D], f32)                # 8KB/p

            # ---------- DRAM collective buffers ----------
            cc_in = dram.tile([CCW], f32)
            cc_out = dram.tile([N_CORES * CCW], f32, addr_space="Shared")

            # ---------- constant + data loads ----------
            nc.gpsimd.dma_start(W1_sb[:], W1_d[:, :])
            nc.gpsimd.dma_start(W2T_sb[:], W2T_d[:, :])
            nc.gpsimd.dma_start(b1_sb[:], b1_d[:, :])
            nc.gpsimd.dma_start(b2_sb[:], b2_d[:, :])
            nc.gpsimd.dma_start(sgn_sb[:], sgn_d[:, :])
            nc.gpsimd.dma_start(lad_sb[:], lad_d[:, :])
            nc.vector.memset(ones_bf[:], 1.0)
            nc.vector.memset(ones_f[:], 1.0)
            nc.vector.memset(fin[:, 3:4], 0.0)
            nc.sync.dma_start(
                lhsT_all[:],
                blockT_d.ap().rearrange("(kt p) m -> p kt m", p=128),
            )

            # ---------- local stats: sq_i (ACT Square) and s (PE) ----------
            # rows are host-permuted so local row index r = p*MT + m lives at
            # partition p, slot m -> the stats DMA out is contiguous
            s_ps = spsum.tile([128, KT], f32, tag="sps")
            for mt in range(MT):
                rt = rowp.tile([128, D], bf16, tag="rt")
                nc.sync.dma_start(rt[:], rows_d[bass.ts(mt, 128), :])
                nc.scalar.activation(
                    junk2[:], rt[:], AF.Square, bias=0.0, scale=1.0,
                    accum_out=rowsq[:, mt : mt + 1],
                )
                for kt in range(KT):
                    nc.tensor.matmul(
                        s_ps[:, kt : kt + 1],
                        rt[:, bass.ts(kt, 128)],
                        ones_bf[:],
                        start=(mt == 0),
                        stop=(mt == MT - 1),
                    )
            nc.scalar.copy(s2d_sb[:], s_ps[:])

            # ---------- one AllGather: [sq_perm (1024) | s (2048)] ----------
            # both input DMAs are contiguous 32/64B lines per partition
            nc.gpsimd.dma_start(
                cc_in[0:RPC].rearrange("(p m) -> p m", p=128), rowsq[:]
            )
            nc.gpsimd.dma_start(
                cc_in[RPC:CCW].rearrange("(p kt) -> p kt", p=128), s2d_sb[:]
            )
            nc.gpsimd.collective_compute(
                "AllGather",
                ALU.bypass,
                replica_groups=rg,
                ins=[cc_in.opt()],
                outs=[cc_out.opt()],
            )
            cc_view = cc_out.rearrange("(c w) -> c w", c=N_CORES)
            # colsq[p, j] = sq_j  (j = c*RPC + r, r contiguous inside block)
            nc.sync.dma_start(
                colsq[:].rearrange("p (c r) -> p c r", c=N_CORES),
                cc_view[None, :, 0:RPC].broadcast_to((128, N_CORES, RPC)),
            )
            # s parts land as [p, c, kt] (contiguous 64B lines per c)
            nc.gpsimd.dma_start(
                s3v[:],
                cc_view[:, RPC:CCW].rearrange("c (p kt) -> p c kt", p=128),
            )

            # ---------- bandwidth ----------
            nc.vector.tensor_reduce(
                s_glob[:], s3v[:].rearrange("p c kt -> p kt c"),
                axis=mybir.AxisListType.X, op=ALU.add,
            )
            nc.vector.tensor_reduce(
                sqsum[:], colsq[0:1, :], axis=mybir.AxisListType.X, op=ALU.add
            )
            nc.vector.tensor_scalar_mul(colsq[:], colsq[:], -0.5)
            nc.vector.tensor_tensor(
                out=sprod[:], in0=s_glob[:], in1=s_glob[:], op=ALU.mult
            )
            nc.vector.tensor_reduce(
                sprod_r[:], sprod[:], axis=mybir.AxisListType.X, op=ALU.add
            )
            s2_ps = spsum.tile([1, 1], f32, tag="sps")
            nc.tensor.matmul(s2_ps[:], sprod_r[:], ones_f[:], start=True, stop=True)
            nc.scalar.copy(s2s[:], s2_ps[:])
            denom = float(NT) * float(NT) - float(NT)
            a_const = float(2.0 * NT / (4.0 * denom))
            b_const = float(-2.0 / (4.0 * denom))
            nc.vector.tensor_scalar_mul(t1s[:], sqsum[:], a_const)
            nc.vector.tensor_scalar(
                out=bws[:], in0=s2s[:], scalar1=b_const, scalar2=t1s[0:1, 0:1],
                op0=ALU.mult, op1=ALU.add,
            )
            nc.vector.reciprocal(bwinv[:], bws[:])
            nc.vector.tensor_scalar_mul(vec5[:], lad_sb[:], bwinv[0:1, 0:1])
            nc.gpsimd.partition_broadcast(vec5b[:], vec5[:])
            nc.vector.tensor_scalar_mul(scales5[:], vec5b[:], 2.0)
            for p in range(KERNEL_NUM):
                nc.vector.tensor_scalar(
                    out=bias_all[:, bass.ts(p, MT)],
                    in0=rowsq[:],
                    scalar1=vec5b[:, p : p + 1],
                    scalar2=-1.0,
                    op0=ALU.mult,
                    op1=ALU.mult,
                )

            # ---------- discriminator collapse + adv partials ----------
            # emitted after the collective so the gpsimd broadcasts can't
            # delay the AllGather trigger
            for ch in range(4):
                w_ps = spsum.tile([1, 512], f32, tag="sps")
                nc.tensor.matmul(
                    w_ps[:], W2T_sb[:], W1_sb[:, bass.ts(ch, 512)],
                    start=True, stop=True,
                )
                nc.scalar.copy(w_sb[:, bass.ts(ch, 512)], w_ps[:])
            beta_ps = spsum.tile([1, 1], f32, tag="sps")
            nc.tensor.matmul(beta_ps[:], W2T_sb[:], b1_sb[:], start=True, stop=True)
            nc.vector.tensor_scalar_add(beta_sb[:], beta_ps[:], b2_sb[0:1, 0:1])
            nc.gpsimd.partition_broadcast(betab[:], beta_sb[:])
            nc.gpsimd.partition_broadcast(wb[:], w_sb[:])
            for mt in range(MT):
                rt2 = rowp.tile([128, D], bf16, tag="rt")
                nc.sync.dma_start(rt2[:], rows_d[bass.ts(mt, 128), :])
                nc.vector.tensor_tensor(
                    out=junk3[:], in0=rt2[:], in1=wb[:], op=ALU.mult
                )
                nc.vector.tensor_reduce(
                    zraw[:, mt : mt + 1], junk3[:],
                    axis=mybir.AxisListType.X, op=ALU.add,
                )
            nc.vector.tensor_scalar_add(zcols[:], zraw[:], betab[:])
            nc.scalar.activation(ecols[:], zcols[:], AF.Exp, bias=0.0, scale=sgn_sb[:])
            nc.scalar.activation(
                lncols[:], ecols[:], AF.Ln, bias=1.0, scale=1.0,
                accum_out=fin[:, 2:3],
            )

            # ---------- main loop ----------
            totalT_r = totalT_d.ap().rearrange("(kt p) n -> p kt n", p=128)
            for n2 in range(NPAIR):
                rhs_a = rhsp.tile([128, KT, NCHUNK], bf16, tag="rhs")
                nc.sync.dma_start(rhs_a[:], totalT_r[:, :, bass.ts(2 * n2, NCHUNK)])
                rhs_b = rhsp.tile([128, KT, NCHUNK], bf16, tag="rhs")
                nc.sync.dma_start(
                    rhs_b[:], totalT_r[:, :, bass.ts(2 * n2 + 1, NCHUNK)]
                )
                for m in range(MT):
                    gt = gpsum.tile([128, 2 * NCHUNK], f32)
                    for k in range(KT):
                        lw = lhsT_all[:, k, bass.ts(m, 128)]
                        nc.tensor.matmul(
                            gt[:, 0:NCHUNK], lw, rhs_a[:, k, :],
                            start=(k == 0), stop=(k == KT - 1),
                        )
                        nc.tensor.matmul(
                            gt[:, NCHUNK : 2 * NCHUNK], lw, rhs_b[:, k, :],
                            start=(k == 0), stop=(k == KT - 1),
                        )
                    # drain PSUM promptly (no collective dependency) ...
                    x2 = x2p.tile([128, 2 * NCHUNK], f32)
                    nc.vector.tensor_copy(x2[:], gt[:])
                    # ... then fold in -sq_j/2 once colsq is available
                    nc.vector.tensor_add(
                        x2[:], x2[:], colsq[:, bass.ts(n2, 2 * NCHUNK)]
                    )
                    for p in range(KERNEL_NUM):
                        ej = ejp.tile([128, 2 * NCHUNK], f32)
                        slot = (n2 * MT + m) * KERNEL_NUM + p
                        nc.scalar.activation(
                            ej[:],
                            x2[:],
                            AF.Exp,
                            bias=bias_all[:, p * MT + m : p * MT + m + 1],
                            scale=scales5[:, p : p + 1],
                            accum_out=accs[:, slot : slot + 1],
                        )

            # ---------- epilogue: block sums -> scalars ----------
            half = (NPAIR // 2) * MT * KERNEL_NUM
            nc.vector.tensor_reduce(
                fin[:, 0:1], accs[:, :half], axis=mybir.AxisListType.X, op=ALU.add
            )
            nc.vector.tensor_reduce(
                fin[:, 1:2], accs[:, half:], axis=mybir.AxisListType.X, op=ALU.add
            )
            fin_ps = spsum.tile([4, 1], f32, tag="sps")
            nc.tensor.matmul(fin_ps[:], fin[:], ones_f[:], start=True, stop=True)
            nc.scalar.copy(outsb[:], fin_ps[:])
            nc.sync.dma_start(out_d[0, 0:4], outsb[:])
            nc.sync.dma_start(out_d[0, 4:5], bws[:])
            nc.sync.dma_start(out_d[0, 5:6], sqsum[:])
            nc.sync.dma_start(out_d[0, 6:7], s2s[:])
            nc.sync.dma_start(out_d[0, 7:8], beta_sb[:])

    nc.compile()
    return nc


def _get_module():
    if "nc" not in _cached:
        _cached["nc"] = _build_module()
    return _cached["nc"]


# permutation inside each 1024-row core block: new local row r' = p*MT + m
# picks old row m*128 + p  (p = partition, m = m-tile slot)
_PERM = np.arange(RPC).reshape(MT, 128).T.reshape(-1)


def kernel(source, target, W1, b1, W2, b2, _trace=False, _trace_kwargs=None):
    import concourse.bass_utils as bass_utils

    nc = _get_module()

    total = np.concatenate(
        [np.asarray(source, np.float32), np.asarray(target, np.float32)], axis=0
    )
    gperm = np.concatenate([c * RPC + _PERM for c in range(N_CORES)])
    tot_bf = total[gperm].astype(ml_dtypes.bfloat16)
    totalT = np.ascontiguousarray(tot_bf.T)  # [D, NT] bf16, shared

    W1f = np.asarray(W1, np.float32)
    W2f = np.asarray(W2, np.float32)
    b1f = np.asarray(b1, np.float32).reshape(128, 1)
    b2f = np.asarray(b2, np.float32).reshape(1, 1)
    W2T = np.ascontiguousarray(W2f.reshape(1, 128).T)
    ladder = (0.5 ** np.arange(KERNEL_NUM, dtype=np.float32)).reshape(1, -1)

    in_maps = []
    for c in range(N_CORES):
        rows = np.ascontiguousarray(tot_bf[c * RPC : (c + 1) * RPC])
        blockT = np.ascontiguousarray(totalT[:, c * RPC : (c + 1) * RPC])
        sgn = np.full((128, 1), -1.0 if c < N_CORES // 2 else 1.0, np.float32)
        in_maps.append(
            {
                "totalT": totalT,
                "blockT": blockT,
                "rows": rows,
                "w1": W1f,
                "w2t": W2T,
                "b1c": b1f,
                "b2c": b2f,
                "sgn": sgn,
                "ladder": ladder,
            }
        )

    kwargs = dict(_trace_kwargs or {})
    res = bass_utils.run_bass_kernel_spmd(
        nc, in_maps, core_ids=list(range(N_CORES)), trace=_trace, **kwargs
    )
    outs = [r["out"][0] for r in res.results]

    SL = [float(o[0]) for o in outs]
    SR = [float(o[1]) for o in outs]
    SA = [float(o[2]) for o in outs]

    h = N_CORES // 2
    sxx = sum(SL[:h])
    syx = sum(SL[h:])
    sxy = sum(SR[:h])
    syy = sum(SR[h:])
    loss = np.float32((sxx + syy - sxy - syx) / (float(B) * float(B)))
    adv = np.float32(sum(SA) / float(NT))

    if _trace:
        kernel._last_results = res
    return (np.asarray(loss, np.float32), np.asarray(adv, np.float32))


# revision 17
# speedup vs baseline: 1.2353x; 1.0972x over previous
"""Adversarial-MMD loss (nn_Advmmd) on 8 Trainium2 NeuronCores via Bass/Tile.

Math (eval mode, lamb=0):
  adv:  the discriminator is Linear(2048,128) -> Dropout(eval) -> Linear(128,1)
        with NO nonlinearity, so it collapses to a single linear functional
        z = x.w + beta with w = W2@W1 [2048], beta = W2@b1 + b2.
        adv_loss = 0.5*(mean log(1+exp(-z_src)) + mean log(1+exp(+z_tgt)))
  mmd:  total = [source;target] [8192,2048]; L2_ij = sq_i + sq_j - 2 G_ij with
        G = total@total.T;  bandwidth bw = sum(L2)/(n^2-n)/4 where
        sum(L2) = 2n*sum(sq) - 2*||sum_j total_j||^2 (exact identity);
        K = sum_{p=0..4} exp(-L2/(bw*2^p));
        loss = mean K[XX] + mean K[YY] - mean K[XY] - mean K[YX].

Distribution: data-parallel over Gram rows.  Core c owns 1024 rows; it
computes its [1024, 8192] Gram block in bf16 on the PE (fp32 accumulate),
applies the five Gaussian kernels on the Scalar engine (exp with
per-partition scale/bias; row sums come free via accum_out) and reduces to
two scalars (left/right half block sums).  Row norms and column sums are
exchanged with a single small AllGather; everything else is local.

Row-tile loads for the stats phase use a strided partition map
(partition p <-> local row p*8+mt) so the per-core stats land contiguously
in DRAM for the collective; all sums are order invariant.

The pipeline drains PE's PSUM through a plain copy into SBUF tiles, so the
matmul stream never waits for the collective; the -sq_j/2 column correction
is added in-place afterwards, once the AllGather lands.
"""

import numpy as np
import ml_dtypes

N_CORES = 8
B = 4096
D = 2048
NT = 2 * B            # 8192 total rows
RPC = NT // N_CORES   # 1024 rows per core
MT = RPC // 128       # 8 m-tiles per core
KT = D // 128         # 16 k-tiles
NCHUNK = 512
NPAIR = NT // (2 * NCHUNK)   # 8 pairs of 512-column chunks
KERNEL_NUM = 5
CCW = RPC + D         # per-core AllGather payload: [sq (1024) | s (2048)]

_cached = {}


def _build_module():
    import concourse.bass as bass
    import concourse.tile as tile
    import concourse.mybir as mybir
    from concourse import bacc

    f32 = mybir.dt.float32
    bf16 = mybir.dt.bfloat16
    AF = mybir.ActivationFunctionType
    ALU = mybir.AluOpType

    nc = bacc.Bacc(
        "TRN2",
        target_bir_lowering=False,
        debug=False,
        enable_asserts=False,
        num_devices=N_CORES,
    )

    totalT_d = nc.dram_tensor("totalT", [D, NT], bf16, kind="ExternalInput")
    blockT_d = nc.dram_tensor("blockT", [D, RPC], bf16, kind="ExternalInput")
    rows_d = nc.dram_tensor("rows", [RPC, D], bf16, kind="ExternalInput")
    W1_d = nc.dram_tensor("w1", [128, D], f32, kind="ExternalInput")
    W2T_d = nc.dram_tensor("w2t", [128, 1], f32, kind="ExternalInput")
    b1_d = nc.dram_tensor("b1c", [128, 1], f32, kind="ExternalInput")
    b2_d = nc.dram_tensor("b2c", [1, 1], f32, kind="ExternalInput")
    sgn_d = nc.dram_tensor("sgn", [128, 1], f32, kind="ExternalInput")
    lad_d = nc.dram_tensor("ladder", [1, KERNEL_NUM], f32, kind="ExternalInput")
    out_d = nc.dram_tensor("out", [1, 8], f32, kind="ExternalOutput")

    rg = [list(range(N_CORES))]

    with tile.TileContext(nc) as tc:
        with (
            tc.tile_pool(name="big", bufs=1) as big,
            tc.tile_pool(name="rhsp", bufs=4) as rhsp,
            tc.tile_pool(name="x2p", bufs=6) as x2p,
            tc.tile_pool(name="ejp", bufs=2) as ejp,
            tc.tile_pool(name="smalls", bufs=1) as smalls,
            tc.tile_pool(name="gpsum", bufs=3, space="PSUM") as gpsum,
            tc.tile_pool(name="spsum", bufs=1, space="PSUM") as spsum,
            tc.tile_pool(name="dram", bufs=1, space="DRAM") as dram,
            tc.tile_pool(name="prol", bufs=1) as prol,
            tc.tile_pool(name="rowp", bufs=2) as rowp,
        ):
            # ---------- persistent tiles ----------
            lhsT_all = big.tile([128, KT, RPC], bf16)       # 32KB/p
            colsq = big.tile([128, NT], f32)                # 32KB/p
            accs = big.tile([128, NPAIR * MT * KERNEL_NUM], f32)
            rowsq = smalls.tile([128, MT], f32)
            bias_all = smalls.tile([128, KERNEL_NUM * MT], f32)
            scales5 = smalls.tile([128, KERNEL_NUM], f32)
            vec5b = smalls.tile([128, KERNEL_NUM], f32)
            zcols = smalls.tile([128, MT], f32)
            zraw = smalls.tile([128, MT], f32)
            ecols = smalls.tile([128, MT], f32)
            lncols = smalls.tile([128, MT], f32)
            fin = smalls.tile([128, 4], f32)
            ones_bf = smalls.tile([128, 1], bf16)
            ones_f = smalls.tile([128, 1], f32)
            sgn_sb = smalls.tile([128, 1], f32)
            lad_sb = smalls.tile([1, KERNEL_NUM], f32)
            vec5 = smalls.tile([1, KERNEL_NUM], f32)
            sqsum = smalls.tile([1, 1], f32)
            s2s = smalls.tile([1, 1], f32)
            t1s = smalls.tile([1, 1], f32)
            bws = smalls.tile([1, 1], f32)
            bwinv = smalls.tile([1, 1], f32)
            beta_sb = smalls.tile([1, 1], f32)
            betab = smalls.tile([128, 1], f32)
            outsb = smalls.tile([4, 1], f32)

            # ---------- prologue tiles ----------
            W1_sb = prol.tile([128, D], f32)                # 8KB/p
            W2T_sb = prol.tile([128, 1], f32)
            b1_sb = prol.tile([128, 1], f32)
            b2_sb = prol.tile([1, 1], f32)
            wb = prol.tile([128, D], bf16)                  # 4KB/p
            w_sb = prol.tile([1, D], bf16)
            s2d_sb = prol.tile([128, KT], f32)
            bias_src = prol.tile([128, MT], f32)
            s3v = prol.tile([128, N_CORES, KT], f32)
            s_glob = prol.tile([128, KT], f32)
            sprod = prol.tile([128, KT], f32)
            sprod_r = prol.tile([128, 1], f32)
            junk2 = prol.tile([128, D], bf16)               # 4KB/p
            junk3 = prol.tile([128, D], f32)                # 8KB/p

            # ---------- DRAM collective buffers ----------
            cc_in = dram.tile([CCW], f32)
            cc_out = dram.tile([N_CORES * CCW], f32, addr_space="Shared")

            # ---------- constant + data loads ----------
            nc.gpsimd.dma_start(W1_sb[:], W1_d[:, :])
            nc.gpsimd.dma_start(W2T_sb[:], W2T_d[:, :])
            nc.gpsimd.dma_start(b1_sb[:], b1_d[:, :])
            nc.gpsimd.dma_start(b2_sb[:], b2_d[:, :])
            nc.gpsimd.dma_start(sgn_sb[:], sgn_d[:, :])
            nc.gpsimd.dma_start(lad_sb[:], lad_d[:, :])
            nc.vector.memset(ones_bf[:], 1.0)
            nc.vector.memset(ones_f[:], 1.0)
            nc.vector.memset(fin[:, 3:4], 0.0)
            nc.sync.dma_start(
                lhsT_all[:],
                blockT_d.ap().rearrange("(kt p) m -> p kt m", p=128),
            )

            # ---------- local stats: sq_i (ACT Square) and s (PE) ----------
            # rows are host-permuted so local row index r = p*MT + m lives at
            # partition p, slot m -> the stats DMA out is contiguous
            s_ps = spsum.tile([128, KT], f32, tag="sps")
            rows_r = rows_d.ap().rearrange("(p m) k -> m p k", m=MT)
            for mt in range(MT):
                rt = rowp.tile([128, D], bf16, tag="rt")
                nc.sync.dma_start(rt[:], rows_r[mt])
                nc.scalar.activation(
                    junk2[:], rt[:], AF.Square, bias=0.0, scale=1.0,
                    accum_out=rowsq[:, mt : mt + 1],
                )
                for kt in range(KT):
                    nc.tensor.matmul(
                        s_ps[:, kt : kt + 1],
                        rt[:, bass.ts(kt, 128)],
                        ones_bf[:],
                        start=(mt == 0),
                        stop=(mt == MT - 1),
                    )
            nc.scalar.copy(s2d_sb[:], s_ps[:])

            # ---------- one AllGather: [sq_perm (1024) | s (2048)] ----------
            # both input DMAs are contiguous 32/64B lines per partition
            nc.gpsimd.dma_start(
                cc_in[0:RPC].rearrange("(p m) -> p m", p=128), rowsq[:]
            )
            nc.gpsimd.dma_start(
                cc_in[RPC:CCW].rearrange("(p kt) -> p kt", p=128), s2d_sb[:]
            )
            nc.gpsimd.collective_compute(
                "AllGather",
                ALU.bypass,
                replica_groups=rg,
                ins=[cc_in.opt()],
                outs=[cc_out.opt()],
            )
            cc_view = cc_out.rearrange("(c w) -> c w", c=N_CORES)
            # colsq[p, j] = sq_j  (j = c*RPC + r, r contiguous inside block)
            nc.scalar.dma_start(
                colsq[:].rearrange("p (c r) -> p c r", c=N_CORES),
                cc_view[None, :, 0:RPC].broadcast_to((128, N_CORES, RPC)),
            )
            # s parts land as [p, c, kt] (contiguous 64B lines per c)
            nc.scalar.dma_start(
                s3v[:],
                cc_view[:, RPC:CCW].rearrange("c (p kt) -> p c kt", p=128),
            )
            # per-(m-tile) bias layout of our own sq, from the local cc input:
            # bias_src[i, m] = sq(row m*128+i)
            nc.scalar.dma_start(
                bias_src[:],
                cc_in[0:RPC].rearrange("(m i) -> i m", i=128),
            )

            # ---------- bandwidth ----------
            nc.vector.tensor_reduce(
                s_glob[:], s3v[:].rearrange("p c kt -> p kt c"),
                axis=mybir.AxisListType.X, op=ALU.add,
            )
            nc.vector.tensor_reduce(
                sqsum[:], colsq[0:1, :], axis=mybir.AxisListType.X, op=ALU.add
            )
            nc.vector.tensor_scalar_mul(colsq[:], colsq[:], -0.5)
            nc.vector.tensor_tensor(
                out=sprod[:], in0=s_glob[:], in1=s_glob[:], op=ALU.mult
            )
            nc.vector.tensor_reduce(
                sprod_r[:], sprod[:], axis=mybir.AxisListType.X, op=ALU.add
            )
            s2_ps = spsum.tile([1, 1], f32, tag="sps")
            nc.tensor.matmul(s2_ps[:], sprod_r[:], ones_f[:], start=True, stop=True)
            nc.scalar.copy(s2s[:], s2_ps[:])
            denom = float(NT) * float(NT) - float(NT)
            a_const = float(2.0 * NT / (4.0 * denom))
            b_const = float(-2.0 / (4.0 * denom))
            nc.vector.tensor_scalar_mul(t1s[:], sqsum[:], a_const)
            nc.vector.tensor_scalar(
                out=bws[:], in0=s2s[:], scalar1=b_const, scalar2=t1s[0:1, 0:1],
                op0=ALU.mult, op1=ALU.add,
            )
            nc.vector.reciprocal(bwinv[:], bws[:])
            nc.vector.tensor_scalar_mul(vec5[:], lad_sb[:], bwinv[0:1, 0:1])
            nc.gpsimd.partition_broadcast(vec5b[:], vec5[:])
            nc.vector.tensor_scalar_mul(scales5[:], vec5b[:], 2.0)
            for p in range(KERNEL_NUM):
                nc.vector.tensor_scalar(
                    out=bias_all[:, bass.ts(p, MT)],
                    in0=bias_src[:],
                    scalar1=vec5b[:, p : p + 1],
                    scalar2=-1.0,
                    op0=ALU.mult,
                    op1=ALU.mult,
                )

            # ---------- discriminator collapse + adv partials ----------
            # emitted after the collective so the gpsimd broadcasts can't
            # delay the AllGather trigger
            for ch in range(4):
                w_ps = spsum.tile([1, 512], f32, tag="sps")
                nc.tensor.matmul(
                    w_ps[:], W2T_sb[:], W1_sb[:, bass.ts(ch, 512)],
                    start=True, stop=True,
                )
                nc.scalar.copy(w_sb[:, bass.ts(ch, 512)], w_ps[:])
            beta_ps = spsum.tile([1, 1], f32, tag="sps")
            nc.tensor.matmul(beta_ps[:], W2T_sb[:], b1_sb[:], start=True, stop=True)
            nc.vector.tensor_scalar_add(beta_sb[:], beta_ps[:], b2_sb[0:1, 0:1])
            nc.gpsimd.partition_broadcast(betab[:], beta_sb[:])
            nc.gpsimd.partition_broadcast(wb[:], w_sb[:])
            for mt in range(MT):
                rt2 = rowp.tile([128, D], bf16, tag="rt")
                nc.sync.dma_start(rt2[:], rows_r[mt])
                nc.vector.tensor_tensor(
                    out=junk3[:], in0=rt2[:], in1=wb[:], op=ALU.mult
                )
                nc.vector.tensor_reduce(
                    zraw[:, mt : mt + 1], junk3[:],
                    axis=mybir.AxisListType.X, op=ALU.add,
                )
            nc.vector.tensor_scalar_add(zcols[:], zraw[:], betab[:])
            nc.scalar.activation(ecols[:], zcols[:], AF.Exp, bias=0.0, scale=sgn_sb[:])
            nc.scalar.activation(
                lncols[:], ecols[:], AF.Ln, bias=1.0, scale=1.0,
                accum_out=fin[:, 2:3],
            )

            # ---------- main loop ----------
            totalT_r = totalT_d.ap().rearrange("(kt p) n -> p kt n", p=128)
            for n2 in range(NPAIR):
                rhs_a = rhsp.tile([128, KT, NCHUNK], bf16, tag="rhs")
                nc.sync.dma_start(rhs_a[:], totalT_r[:, :, bass.ts(2 * n2, NCHUNK)])
                rhs_b = rhsp.tile([128, KT, NCHUNK], bf16, tag="rhs")
                nc.sync.dma_start(
                    rhs_b[:], totalT_r[:, :, bass.ts(2 * n2 + 1, NCHUNK)]
                )
                for m in range(MT):
                    gt = gpsum.tile([128, 2 * NCHUNK], f32)
                    for k in range(KT):
                        lw = lhsT_all[:, k, bass.ts(m, 128)]
                        nc.tensor.matmul(
                            gt[:, 0:NCHUNK], lw, rhs_a[:, k, :],
                            start=(k == 0), stop=(k == KT - 1),
                        )
                        nc.tensor.matmul(
                            gt[:, NCHUNK : 2 * NCHUNK], lw, rhs_b[:, k, :],
                            start=(k == 0), stop=(k == KT - 1),
                        )
                    # drain PSUM promptly (no collective dependency) ...
                    x2 = x2p.tile([128, 2 * NCHUNK], f32)
                    nc.vector.tensor_copy(x2[:], gt[:])
                    # ... then fold in -sq_j/2 once colsq is available
                    nc.vector.tensor_add(
                        x2[:], x2[:], colsq[:, bass.ts(n2, 2 * NCHUNK)]
                    )
                    for p in range(KERNEL_NUM):
                        ej = ejp.tile([128, 2 * NCHUNK], f32)
                        slot = (n2 * MT + m) * KERNEL_NUM + p
                        nc.scalar.activation(
                            ej[:],
                            x2[:],
                            AF.Exp,
                            bias=bias_all[:, p * MT + m : p * MT + m + 1],
                            scale=scales5[:, p : p + 1],
                            accum_out=accs[:, slot : slot + 1],
                        )

            # ---------- epilogue: block sums -> scalars ----------
            half = (NPAIR // 2) * MT * KERNEL_NUM
            nc.vector.tensor_reduce(
                fin[:, 0:1], accs[:, :half], axis=mybir.AxisListType.X, op=ALU.add
            )
            nc.vector.tensor_reduce(
                fin[:, 1:2], accs[:, half:], axis=mybir.AxisListType.X, op=ALU.add
            )
            fin_ps = spsum.tile([4, 1], f32, tag="sps")
            nc.tensor.matmul(fin_ps[:], fin[:], ones_f[:], start=True, stop=True)
            nc.scalar.copy(outsb[:], fin_ps[:])
            nc.sync.dma_start(out_d[0, 0:4], outsb[:])
            nc.sync.dma_start(out_d[0, 4:5], bws[:])
            nc.sync.dma_start(out_d[0, 5:6], sqsum[:])
            nc.sync.dma_start(out_d[0, 6:7], s2s[:])
            nc.sync.dma_start(out_d[0, 7:8], beta_sb[:])

    nc.compile()
    return nc


def _get_module():
    if "nc" not in _cached:
        _cached["nc"] = _build_module()
    return _cached["nc"]


def kernel(source, target, W1, b1, W2, b2, _trace=False, _trace_kwargs=None):
    import concourse.bass_utils as bass_utils

    nc = _get_module()

    total = np.concatenate(
        [np.asarray(source, np.float32), np.asarray(target, np.float32)], axis=0
    )
    tot_bf = total.astype(ml_dtypes.bfloat16)
    totalT = np.ascontiguousarray(tot_bf.T)  # [D, NT] bf16, shared

    W1f = np.asarray(W1, np.float32)
    W2f = np.asarray(W2, np.float32)
    b1f = np.asarray(b1, np.float32).reshape(128, 1)
    b2f = np.asarray(b2, np.float32).reshape(1, 1)
    W2T = np.ascontiguousarray(W2f.reshape(1, 128).T)
    ladder = (0.5 ** np.arange(KERNEL_NUM, dtype=np.float32)).reshape(1, -1)

    in_maps = []
    for c in range(N_CORES):
        rows = np.ascontiguousarray(tot_bf[c * RPC : (c + 1) * RPC])
        blockT = np.ascontiguousarray(totalT[:, c * RPC : (c + 1) * RPC])
        sgn = np.full((128, 1), -1.0 if c < N_CORES // 2 else 1.0, np.float32)
        in_maps.append(
            {
                "totalT": totalT,
                "blockT": blockT,
                "rows": rows,
                "w1": W1f,
                "w2t": W2T,
                "b1c": b1f,
                "b2c": b2f,
                "sgn": sgn,
                "ladder": ladder,
            }
        )

    kwargs = dict(_trace_kwargs or {})
    res = bass_utils.run_bass_kernel_spmd(
        nc, in_maps, core_ids=list(range(N_CORES)), trace=_trace, **kwargs
    )
    outs = [r["out"][0] for r in res.results]

    SL = [float(o[0]) for o in outs]
    SR = [float(o[1]) for o in outs]
    SA = [float(o[2]) for o in outs]

    h = N_CORES // 2
    sxx = sum(SL[:h])
    syx = sum(SL[h:])
    sxy = sum(SR[:h])
    syy = sum(SR[h:])
    loss = np.float32((sxx + syy - sxy - syx) / (float(B) * float(B)))
    adv = np.float32(sum(SA) / float(NT))

    if _trace:
        kernel._last_results = res
    return (np.asarray(loss, np.float32), np.asarray(adv, np.float32))


# revision 18
# speedup vs baseline: 1.5712x; 1.2719x over previous
"""Adversarial-MMD loss (nn_Advmmd) on 8 Trainium2 NeuronCores via Bass/Tile.

Math (eval mode, lamb=0):
  adv:  the discriminator is Linear(2048,128) -> Dropout(eval) -> Linear(128,1)
        with NO nonlinearity, so it collapses to a single linear functional
        z = x.w + beta with w = W2@W1 [2048], beta = W2@b1 + b2.
        adv_loss = 0.5*(mean log(1+exp(-z_src)) + mean log(1+exp(+z_tgt)))
  mmd:  total = [source;target] [8192,2048]; L2_ij = sq_i + sq_j - 2 G_ij with
        G = total@total.T;  bandwidth bw = sum(L2)/(n^2-n)/4 where
        sum(L2) = 2n*sum(sq) - 2*||sum_j total_j||^2 (exact identity);
        K = sum_{p=0..4} exp(-L2/(bw*2^p));
        loss = mean K[XX] + mean K[YY] - mean K[XY] - mean K[YX].

Distribution: data-parallel over Gram rows.  Core c owns 1024 rows; it
computes its [1024, 8192] Gram block in bf16 on the PE (fp32 accumulate),
applies the five Gaussian kernels on the Scalar engine (exp with
per-partition scale/bias; row sums come free via accum_out) and reduces to
two scalars (left/right half block sums).  Row norms and column sums are
exchanged with a single small AllGather; everything else is local.

Row-tile loads for the stats phase use a strided partition map
(partition p <-> local row p*8+mt) so the per-core stats land contiguously
in DRAM for the collective; all sums are order invariant.

The pipeline drains PE's PSUM through a plain copy into SBUF tiles, so the
matmul stream never waits for the collective; the -sq_j/2 column correction
is added in-place afterwards, once the AllGather lands.
"""

import numpy as np
import ml_dtypes

N_CORES = 8
B = 4096
D = 2048
NT = 2 * B            # 8192 total rows
RPC = NT // N_CORES   # 1024 rows per core
MT = RPC // 128       # 8 m-tiles per core
KT = D // 128         # 16 k-tiles
NCHUNK = 512
NQUAD = NT // (4 * NCHUNK)   # 4 groups of four 512-column chunks
KP = KT // 2                 # 8 double-row k-pairs
KERNEL_NUM = 5
CCW = RPC + D         # per-core AllGather payload: [sq (1024) | s (2048)]

_cached = {}


def _build_module():
    import concourse.bass as bass
    import concourse.tile as tile
    import concourse.mybir as mybir
    from concourse import bacc

    f32 = mybir.dt.float32
    bf16 = mybir.dt.bfloat16
    AF = mybir.ActivationFunctionType
    ALU = mybir.AluOpType

    nc = bacc.Bacc(
        "TRN2",
        target_bir_lowering=False,
        debug=False,
        enable_asserts=False,
        num_devices=N_CORES,
    )

    fp8 = mybir.dt.float8e4
    totalT_d = nc.dram_tensor("totalT", [D, NT], fp8, kind="ExternalInput")
    blockT_d = nc.dram_tensor("blockT", [D, RPC], fp8, kind="ExternalInput")
    rows_d = nc.dram_tensor("rows", [RPC, D], fp8, kind="ExternalInput")
    rowsbf_d = nc.dram_tensor("rowsbf", [RPC, D], bf16, kind="ExternalInput")
    W1_d = nc.dram_tensor("w1", [128, D], f32, kind="ExternalInput")
    W2T_d = nc.dram_tensor("w2t", [128, 1], f32, kind="ExternalInput")
    b1_d = nc.dram_tensor("b1c", [128, 1], f32, kind="ExternalInput")
    b2_d = nc.dram_tensor("b2c", [1, 1], f32, kind="ExternalInput")
    sgn_d = nc.dram_tensor("sgn", [128, 1], f32, kind="ExternalInput")
    lad_d = nc.dram_tensor("ladder", [1, KERNEL_NUM], f32, kind="ExternalInput")
    out_d = nc.dram_tensor("out", [1, 8], f32, kind="ExternalOutput")

    rg = [list(range(N_CORES))]

    with tile.TileContext(nc) as tc:
        with (
            tc.tile_pool(name="big", bufs=1) as big,
            tc.tile_pool(name="rhsp", bufs=6) as rhsp,
            tc.tile_pool(name="x2p", bufs=5) as x2p,
            tc.tile_pool(name="ejp", bufs=2) as ejp,
            tc.tile_pool(name="smalls", bufs=1) as smalls,
            tc.tile_pool(name="gpsum", bufs=3, space="PSUM") as gpsum,
            tc.tile_pool(name="spsum", bufs=1, space="PSUM") as spsum,
            tc.tile_pool(name="dram", bufs=1, space="DRAM") as dram,
            tc.tile_pool(name="prol", bufs=1) as prol,
            tc.tile_pool(name="rowp", bufs=2) as rowp,
        ):
            # ---------- persistent tiles ----------
            lhsT_all = big.tile([128, KP, 2, RPC], fp8)     # 16KB/p
            colsq = big.tile([128, NT], f32)                # 32KB/p
            accs = big.tile([128, NQUAD * MT * KERNEL_NUM], f32)
            rowsq = smalls.tile([128, MT], f32)
            bias_all = smalls.tile([128, KERNEL_NUM * MT], f32)
            scales5 = smalls.tile([128, KERNEL_NUM], f32)
            vec5b = smalls.tile([128, KERNEL_NUM], f32)
            zcols = smalls.tile([128, MT], f32)
            zraw = smalls.tile([128, MT], f32)
            ecols = smalls.tile([128, MT], f32)
            lncols = smalls.tile([128, MT], f32)
            fin = smalls.tile([128, 4], f32)
            ones_8 = smalls.tile([128, 1], fp8)
            ones_f = smalls.tile([128, 1], f32)
            sgn_sb = smalls.tile([128, 1], f32)
            lad_sb = smalls.tile([1, KERNEL_NUM], f32)
            vec5 = smalls.tile([1, KERNEL_NUM], f32)
            sqsum = smalls.tile([1, 1], f32)
            s2s = smalls.tile([1, 1], f32)
            t1s = smalls.tile([1, 1], f32)
            bws = smalls.tile([1, 1], f32)
            bwinv = smalls.tile([1, 1], f32)
            beta_sb = smalls.tile([1, 1], f32)
            betab = smalls.tile([128, 1], f32)
            outsb = smalls.tile([4, 1], f32)

            # ---------- prologue tiles ----------
            W1_sb = prol.tile([128, D], f32)                # 8KB/p
            W2T_sb = prol.tile([128, 1], f32)
            b1_sb = prol.tile([128, 1], f32)
            b2_sb = prol.tile([1, 1], f32)
            wb = prol.tile([128, D], bf16)                  # 4KB/p
            w_sb = prol.tile([1, D], bf16)
            s2d_sb = prol.tile([128, KT], f32)
            bias_src = prol.tile([128, MT], f32)
            s3v = prol.tile([128, N_CORES, KT], f32)
            s_glob = prol.tile([128, KT], f32)
            sprod = prol.tile([128, KT], f32)
            sprod_r = prol.tile([128, 1], f32)
            junk2 = prol.tile([128, D], bf16)               # 4KB/p
            junk3 = prol.tile([128, D], f32)                # 8KB/p

            # ---------- DRAM collective buffers ----------
            cc_in = dram.tile([CCW], f32)
            cc_out = dram.tile([N_CORES * CCW], f32, addr_space="Shared")

            # ---------- constant + data loads ----------
            nc.gpsimd.dma_start(W1_sb[:], W1_d[:, :])
            nc.gpsimd.dma_start(W2T_sb[:], W2T_d[:, :])
            nc.gpsimd.dma_start(b1_sb[:], b1_d[:, :])
            nc.gpsimd.dma_start(b2_sb[:], b2_d[:, :])
            nc.gpsimd.dma_start(sgn_sb[:], sgn_d[:, :])
            nc.gpsimd.dma_start(lad_sb[:], lad_d[:, :])
            nc.vector.memset(ones_8[:], 1.0)
            nc.vector.memset(ones_f[:], 1.0)
            nc.vector.memset(fin[:, 3:4], 0.0)
            nc.sync.dma_start(
                lhsT_all[:],
                blockT_d.ap().rearrange("(kp two p) m -> p kp two m", p=128, two=2),
            )

            # ---------- local stats: sq_i (ACT Square) and s (PE) ----------
            # rows are host-permuted so local row index r = p*MT + m lives at
            # partition p, slot m -> the stats DMA out is contiguous
            s_ps = spsum.tile([128, KT], f32, tag="sps")
            rows_r = rows_d.ap().rearrange("(p m) k -> m p k", m=MT)
            rowsbf_r = rowsbf_d.ap().rearrange("(p m) k -> m p k", m=MT)
            for mt in range(MT):
                rt = rowp.tile([128, D], fp8, tag="rt8")
                nc.sync.dma_start(rt[:], rows_r[mt])
                nc.scalar.activation(
                    junk2[:], rt[:], AF.Square, bias=0.0, scale=1.0,
                    accum_out=rowsq[:, mt : mt + 1],
                )
                for kt in range(KT):
                    nc.tensor.matmul(
                        s_ps[:, kt : kt + 1],
                        rt[:, bass.ts(kt, 128)],
                        ones_8[:],
                        start=(mt == 0),
                        stop=(mt == MT - 1),
                    )
            nc.scalar.copy(s2d_sb[:], s_ps[:])

            # ---------- one AllGather: [sq_perm (1024) | s (2048)] ----------
            # both input DMAs are contiguous 32/64B lines per partition
            nc.gpsimd.dma_start(
                cc_in[0:RPC].rearrange("(p m) -> p m", p=128), rowsq[:]
            )
            nc.gpsimd.dma_start(
                cc_in[RPC:CCW].rearrange("(p kt) -> p kt", p=128), s2d_sb[:]
            )
            nc.gpsimd.collective_compute(
                "AllGather",
                ALU.bypass,
                replica_groups=rg,
                ins=[cc_in.opt()],
                outs=[cc_out.opt()],
            )
            cc_view = cc_out.rearrange("(c w) -> c w", c=N_CORES)
            # colsq[p, j] = sq_j  (j = c*RPC + r, r contiguous inside block)
            nc.scalar.dma_start(
                colsq[:].rearrange("p (c r) -> p c r", c=N_CORES),
                cc_view[None, :, 0:RPC].broadcast_to((128, N_CORES, RPC)),
            )
            # s parts land as [p, c, kt] (contiguous 64B lines per c)
            nc.scalar.dma_start(
                s3v[:],
                cc_view[:, RPC:CCW].rearrange("c (p kt) -> p c kt", p=128),
            )
            # per-(m-tile) bias layout of our own sq, from the local cc input:
            # bias_src[i, m] = sq(row m*128+i)
            nc.scalar.dma_start(
                bias_src[:],
                cc_in[0:RPC].rearrange("(m i) -> i m", i=128),
            )

            # ---------- bandwidth ----------
            nc.vector.tensor_reduce(
                s_glob[:], s3v[:].rearrange("p c kt -> p kt c"),
                axis=mybir.AxisListType.X, op=ALU.add,
            )
            nc.vector.tensor_reduce(
                sqsum[:], colsq[0:1, :], axis=mybir.AxisListType.X, op=ALU.add
            )
            nc.vector.tensor_scalar_mul(colsq[:], colsq[:], -0.5)
            nc.vector.tensor_tensor(
                out=sprod[:], in0=s_glob[:], in1=s_glob[:], op=ALU.mult
            )
            nc.vector.tensor_reduce(
                sprod_r[:], sprod[:], axis=mybir.AxisListType.X, op=ALU.add
            )
            s2_ps = spsum.tile([1, 1], f32, tag="sps")
            nc.tensor.matmul(s2_ps[:], sprod_r[:], ones_f[:], start=True, stop=True)
            nc.scalar.copy(s2s[:], s2_ps[:])
            denom = float(NT) * float(NT) - float(NT)
            a_const = float(2.0 * NT / (4.0 * denom))
            b_const = float(-2.0 / (4.0 * denom))
            nc.vector.tensor_scalar_mul(t1s[:], sqsum[:], a_const)
            nc.vector.tensor_scalar(
                out=bws[:], in0=s2s[:], scalar1=b_const, scalar2=t1s[0:1, 0:1],
                op0=ALU.mult, op1=ALU.add,
            )
            nc.vector.reciprocal(bwinv[:], bws[:])
            nc.vector.tensor_scalar_mul(vec5[:], lad_sb[:], bwinv[0:1, 0:1])
            nc.gpsimd.partition_broadcast(vec5b[:], vec5[:])
            nc.vector.tensor_scalar_mul(scales5[:], vec5b[:], 2.0)
            for p in range(KERNEL_NUM):
                nc.vector.tensor_scalar(
                    out=bias_all[:, bass.ts(p, MT)],
                    in0=bias_src[:],
                    scalar1=vec5b[:, p : p + 1],
                    scalar2=-1.0,
                    op0=ALU.mult,
                    op1=ALU.mult,
                )

            # ---------- discriminator collapse + adv partials ----------
            # emitted after the collective so the gpsimd broadcasts can't
            # delay the AllGather trigger
            for ch in range(4):
                w_ps = spsum.tile([1, 512], f32, tag="sps")
                nc.tensor.matmul(
                    w_ps[:], W2T_sb[:], W1_sb[:, bass.ts(ch, 512)],
                    start=True, stop=True,
                )
                nc.scalar.copy(w_sb[:, bass.ts(ch, 512)], w_ps[:])
            beta_ps = spsum.tile([1, 1], f32, tag="sps")
            nc.tensor.matmul(beta_ps[:], W2T_sb[:], b1_sb[:], start=True, stop=True)
            nc.vector.tensor_scalar_add(beta_sb[:], beta_ps[:], b2_sb[0:1, 0:1])
            nc.gpsimd.partition_broadcast(betab[:], beta_sb[:])
            nc.gpsimd.partition_broadcast(wb[:], w_sb[:])
            for mt in range(MT):
                rt2 = rowp.tile([128, D], bf16, tag="rt")
                nc.sync.dma_start(rt2[:], rowsbf_r[mt])
                nc.vector.tensor_tensor(
                    out=junk3[:], in0=rt2[:], in1=wb[:], op=ALU.mult
                )
                nc.vector.tensor_reduce(
                    zraw[:, mt : mt + 1], junk3[:],
                    axis=mybir.AxisListType.X, op=ALU.add,
                )
            nc.vector.tensor_scalar_add(zcols[:], zraw[:], betab[:])
            nc.scalar.activation(ecols[:], zcols[:], AF.Exp, bias=0.0, scale=sgn_sb[:])
            nc.scalar.activation(
                lncols[:], ecols[:], AF.Ln, bias=1.0, scale=1.0,
                accum_out=fin[:, 2:3],
            )

            # ---------- main loop ----------
            totalT_r = totalT_d.ap().rearrange(
                "(kp two p) n -> p kp two n", p=128, two=2
            )
            DR = mybir.MatmulPerfMode.DoubleRow
            for n4 in range(NQUAD):
                rq = []
                for q in range(4):
                    rqt = rhsp.tile([128, KP, 2, NCHUNK], fp8, tag="rhs")
                    nc.sync.dma_start(
                        rqt[:], totalT_r[:, :, :, bass.ts(4 * n4 + q, NCHUNK)]
                    )
                    rq.append(rqt)
                for m in range(MT):
                    x2 = x2p.tile([128, 4 * NCHUNK], f32)
                    for half in range(2):
                        gt = gpsum.tile([128, 2 * NCHUNK], f32)
                        for kp in range(KP):
                            lw = lhsT_all[:, kp, :, bass.ts(m, 128)]
                            nc.tensor.matmul(
                                gt[:, 0:NCHUNK],
                                lw,
                                rq[2 * half][:, kp, :, :],
                                start=(kp == 0), stop=(kp == KP - 1),
                                perf_mode=DR,
                            )
                            nc.tensor.matmul(
                                gt[:, NCHUNK : 2 * NCHUNK],
                                lw,
                                rq[2 * half + 1][:, kp, :, :],
                                start=(kp == 0), stop=(kp == KP - 1),
                                perf_mode=DR,
                            )
                        nc.vector.tensor_copy(
                            x2[:, half * 1024 : (half + 1) * 1024], gt[:]
                        )
                    nc.vector.tensor_add(
                        x2[:], x2[:], colsq[:, bass.ts(n4, 4 * NCHUNK)]
                    )
                    for p in range(KERNEL_NUM):
                        ej = ejp.tile([128, 4 * NCHUNK], bf16)
                        slot = (n4 * MT + m) * KERNEL_NUM + p
                        nc.scalar.activation(
                            ej[:],
                            x2[:],
                            AF.Exp,
                            bias=bias_all[:, p * MT + m : p * MT + m + 1],
                            scale=scales5[:, p : p + 1],
                            accum_out=accs[:, slot : slot + 1],
                        )

            # ---------- epilogue: block sums -> scalars ----------
            half = (NQUAD // 2) * MT * KERNEL_NUM
            nc.vector.tensor_reduce(
                fin[:, 0:1], accs[:, :half], axis=mybir.AxisListType.X, op=ALU.add
            )
            nc.vector.tensor_reduce(
                fin[:, 1:2], accs[:, half:], axis=mybir.AxisListType.X, op=ALU.add
            )
            fin_ps = spsum.tile([4, 1], f32, tag="sps")
            nc.tensor.matmul(fin_ps[:], fin[:], ones_f[:], start=True, stop=True)
            nc.scalar.copy(outsb[:], fin_ps[:])
            nc.sync.dma_start(out_d[0, 0:4], outsb[:])
            nc.sync.dma_start(out_d[0, 4:5], bws[:])
            nc.sync.dma_start(out_d[0, 5:6], sqsum[:])
            nc.sync.dma_start(out_d[0, 6:7], s2s[:])
            nc.sync.dma_start(out_d[0, 7:8], beta_sb[:])

    nc.compile()
    return nc


def _get_module():
    if "nc" not in _cached:
        _cached["nc"] = _build_module()
    return _cached["nc"]


def kernel(source, target, W1, b1, W2, b2, _trace=False, _trace_kwargs=None):
    import concourse.bass_utils as bass_utils

    nc = _get_module()

    total = np.concatenate(
        [np.asarray(source, np.float32), np.asarray(target, np.float32)], axis=0
    )
    tot8 = total.astype(ml_dtypes.float8_e4m3)
    tot_bf = total.astype(ml_dtypes.bfloat16)
    totalT = np.ascontiguousarray(tot8.T)  # [D, NT] fp8, shared

    W1f = np.asarray(W1, np.float32)
    W2f = np.asarray(W2, np.float32)
    b1f = np.asarray(b1, np.float32).reshape(128, 1)
    b2f = np.asarray(b2, np.float32).reshape(1, 1)
    W2T = np.ascontiguousarray(W2f.reshape(1, 128).T)
    ladder = (0.5 ** np.arange(KERNEL_NUM, dtype=np.float32)).reshape(1, -1)

    in_maps = []
    for c in range(N_CORES):
        rows = np.ascontiguousarray(tot8[c * RPC : (c + 1) * RPC])
        rowsbf = np.ascontiguousarray(tot_bf[c * RPC : (c + 1) * RPC])
        blockT = np.ascontiguousarray(totalT[:, c * RPC : (c + 1) * RPC])
        sgn = np.full((128, 1), -1.0 if c < N_CORES // 2 else 1.0, np.float32)
        in_maps.append(
            {
                "totalT": totalT,
                "blockT": blockT,
                "rows": rows,
                "rowsbf": rowsbf,
                "w1": W1f,
                "w2t": W2T,
                "b1c": b1f,
                "b2c": b2f,
                "sgn": sgn,
                "ladder": ladder,
            }
        )

    kwargs = dict(_trace_kwargs or {})
    res = bass_utils.run_bass_kernel_spmd(
        nc, in_maps, core_ids=list(range(N_CORES)), trace=_trace, **kwargs
    )
    outs = [r["out"][0] for r in res.results]

    SL = [float(o[0]) for o in outs]
    SR = [float(o[1]) for o in outs]
    SA = [float(o[2]) for o in outs]

    h = N_CORES // 2
    sxx = sum(SL[:h])
    syx = sum(SL[h:])
    sxy = sum(SR[:h])
    syy = sum(SR[h:])
    loss = np.float32((sxx + syy - sxy - syx) / (float(B) * float(B)))
    adv = np.float32(sum(SA) / float(NT))

    if _trace:
        kernel._last_results = res
    return (np.asarray(loss, np.float32), np.asarray(adv, np.float32))
